# revision 62
# baseline (speedup 1.0000x reference)
"""Trainium2 Bass kernel for single-step causal GQA attention with KV cache.

Problem (hardcoded shapes):
  x[4,16,4096] @ Wq/Wk/Wv -> RoPE -> append to KV cache (start_pos=2048) ->
  GQA attention over T=2064 keys -> @ Wo -> out[4,16,4096], fp32 in/out.

Sharding (8 cores, tensor-parallel over heads):
  core c gets q-heads 4c..4c+3 (Wq cols c*512:(c+1)*512), kv-head c
  (Wk/Wv cols c*128:(c+1)*128, cache [:, :, c, :]), and Wo rows
  c*512:(c+1)*512 (row-parallel). The 8 partial [64,4096] outputs are
  summed on the host.

The kernel is HBM-bound, so all large operands (weights, caches, x, the
attention intermediates, and the partial outputs) are bf16 — converted on
the host, halving DMA bytes. Accumulation stays fp32 in PSUM; measured
rel err ~1e-3 vs the 2e-2 gate.

Per-core layout choices:
  - x pre-transposed on host to xT[128, KC*64] (SBUF order, contiguous).
  - Wk|Wv interleaved per contraction chunk (wkv) so one matmul per chunk
    produces xk and xv side by side in one PSUM tile; wkv loads before wq
    so the new-token k/v path (RoPE-k, DRAM bounce of v) finishes early.
  - K cache pre-transposed to kT[4,128,2048]; score matmuls use kT chunks
    as stationary, scores come out transposed (keys on partitions).
  - V cache stored as 16 chunks of [128, 129] per batch with a ones
    column baked in at col 128 (host-side), so the softmax denominator
    accumulates next to the attention output in the same matmuls and the
    whole V load is one contiguous DMA.
  - softmax: exp without max-subtraction (|scores|<~10 here, exp exact in
    fp32), denominator via the ones column; no cross-partition reduction.
  - RoPE: two multiplies and an add with one host-precomputed [64,128]
    cos/sin pair (32KB load) serving k directly and q via a zero-stride
    AP repeating it across the 4 heads; pair-swap via negative-stride AP.
  - The new-token V is regrouped to per-batch layout with two PE
    transposes instead of a DRAM bounce (no extra stream bytes).
  - DMA count kept low (~30: each dma_start costs ~625ns on the
    serialized HWDGE path) and all loads issue on SP in consumption
    order so the DMA stream is gapless; Wo loads go progressively finer
    (pairs -> singles -> halves) so the tail of the stream has the
    shortest dependent chain, and the final out store is one small DMA
    whose wait covers only the last few columns.
"""

import math

import numpy as np
import ml_dtypes

import concourse.bass as bass
import concourse.mybir as mybir
import concourse.tile as tile
from concourse import bacc
from concourse.bass_utils import run_bass_kernel_spmd
from concourse.masks import make_identity

F32 = mybir.dt.float32
BF16 = mybir.dt.bfloat16
I16 = mybir.dt.int16
NPBF16 = ml_dtypes.bfloat16

B, S, DIM = 4, 16, 4096
NH, NKV, HD = 32, 8, 128
START = 2048
BS = B * S              # 64 tokens
NCORES = 8
QH = NH // NCORES       # 4 q heads per core
QW = QH * HD            # 512 = per-core Wq width
KC = DIM // 128         # 32 contraction chunks
NT = START // 128       # 16 full cache chunks per batch
TW = QH * S             # 64 = scoresT free width (h-major, then s)
EXPW = NT * TW + TW     # 1088 = expT tile width (16 cache chunks + new chunk)
VW = HD + 1             # 129 = V chunk stride (ones col at 128)
SCALE = 1.0 / math.sqrt(HD)


def swap_pairs(ap):
    # [p, w] view with even/odd element pairs swapped: (x1,x0,x3,x2,...)
    p, w = ap.shape
    return bass.AP(ap.tensor, ap.offset + 1, [ap.ap[0], [2, w // 2], [-1, 2]])


def build_bass() -> bass.Bass:
    nc = bacc.Bacc()

    xT = nc.dram_tensor("xT", [128, KC * BS], BF16, kind="ExternalInput")
    rope = nc.dram_tensor("rope", [BS, 2 * HD], BF16, kind="ExternalInput")
    wq = nc.dram_tensor("wq", [128, KC * QW], BF16, kind="ExternalInput")
    wkv = nc.dram_tensor("wkv", [128, KC * 2 * HD], BF16, kind="ExternalInput")
    kT = nc.dram_tensor("kT", [B, HD, START], BF16, kind="ExternalInput")
    vc = nc.dram_tensor("vc", [B, 128, NT * VW], BF16, kind="ExternalInput")
    wo = nc.dram_tensor("wo", [128, 8, 4 * 512], BF16, kind="ExternalInput")
    out = nc.dram_tensor("out", [BS, DIM], BF16, kind="ExternalOutput")

    with tile.TileContext(nc) as tc:
        with (
            tc.tile_pool(name="const", bufs=1) as const,
            tc.tile_pool(name="wqp", bufs=4) as wqp,
            tc.tile_pool(name="wkvp", bufs=1) as wkvp,
            tc.tile_pool(name="wop", bufs=2) as wop,
            tc.tile_pool(name="wop2", bufs=4) as wop2,
            tc.tile_pool(name="kvp", bufs=4) as kvp,
            tc.tile_pool(name="acts", bufs=1) as acts,
            tc.tile_pool(name="expp", bufs=2) as expp,
            tc.tile_pool(name="small", bufs=4) as small,
        ):
            ident = const.tile([128, 128], F32, tag="ident")
            make_identity(nc, ident[:])
            # touch Exp once so the ACT LUT loads during phase 1, not on
            # the batch-0 softmax critical path
            warm = const.tile([1, 4], F32, tag="warm")
            nc.scalar.activation(
                warm[:], ident[:1, :4], mybir.ActivationFunctionType.Exp
            )

            xT_sb = const.tile([128, KC, BS], BF16, tag="xT")
            rope_sb = const.tile([BS, 2 * HD], BF16, tag="rope")
            # one [64,128] cos/sin pair serves k directly and q via a
            # zero-stride AP that repeats it across the 4 heads
            cck = rope_sb[:, :HD]
            ssk = rope_sb[:, HD:]
            ccq = bass.AP(cck.tensor, cck.offset, [cck.ap[0], [0, QH], [1, HD]])
            ssq = bass.AP(ssk.tensor, ssk.offset, [ssk.ap[0], [0, QH], [1, HD]])

            wo_sb = []
            attnT = acts.tile([128, QH, BS], BF16, tag="attnT")

            with tc.tile_pool(name="ps_t", bufs=2, space="PSUM") as ps_t:
                # ---- QKV projections: psum[tok, width] += xT_c.T @ W_c ----
                with tc.tile_pool(name="ps_qkv", bufs=1, space="PSUM") as ps_qkv:
                    xq_ps = ps_qkv.tile([BS, QW], F32, tag="xq")
                    xkv_ps = ps_qkv.tile([BS, 2 * HD], F32, tag="xkv")

                    # Load order: wkv before wq, so the k/v new-token path
                    # (RoPE-k, DRAM bounce of v) completes early and never
                    # blocks the SP DMA queue; wq streams after.
                    x4 = xT.ap()
                    wq3 = wq.ap()
                    wkv2 = wkv.ap()
                    wkv_sb = wkvp.tile([128, KC, 2 * HD], BF16, tag="wkv")
                    # big wkv half first: its 2.9us transfer hides the SP
                    # issue cadence (~650ns/DMA) for the small loads behind
                    nc.sync.dma_start(wkv_sb[:, :16, :], wkv2[:, : 16 * 256])
                    nc.sync.dma_start(xT_sb[:, :8, :], x4[:, :512])
                    nc.sync.dma_start(rope_sb[:], rope.ap())
                    nc.sync.dma_start(xT_sb[:, 8:16, :], x4[:, 512:1024])
                    nc.sync.dma_start(xT_sb[:, 16:24, :], x4[:, 1024:1536])
                    nc.sync.dma_start(wkv_sb[:, 16:, :], wkv2[:, 16 * 256 :])
                    nc.sync.dma_start(xT_sb[:, 24:, :], x4[:, 1536:])
                    wq_tiles = []
                    for g in range(4):
                        wq_sb = wqp.tile([128, 8, QW], BF16, tag="wq")
                        nc.sync.dma_start(
                            wq_sb[:], wq3[:, 4096 * g : 4096 * (g + 1)]
                        )
                        wq_tiles.append(wq_sb)

                    for c in range(KC):
                        nc.tensor.matmul(
                            xkv_ps[:],
                            lhsT=xT_sb[:, c, :],
                            rhs=wkv_sb[:, c, :],
                            start=(c == 0),
                            stop=(c == KC - 1),
                        )

                    # ---- RoPE k + v_new bounce (early, off the SP queue) --
                    xk_sb = acts.tile([BS, HD], F32, tag="xk_sb")
                    tk = acts.tile([BS, HD], F32, tag="tk")
                    xk_ps = xkv_ps[:, :HD]
                    nc.vector.tensor_mul(xk_sb[:], xk_ps, cck)
                    nc.vector.tensor_mul(tk[:], swap_pairs(xk_ps), ssk)
                    nc.vector.tensor_add(xk_sb[:], xk_sb[:], tk[:])

                    # v_new: regroup the token-major new-token V to
                    # per-batch [16(s), b, HD] via a double PE transpose
                    # (token-major -> hd-major -> per-batch slices), with a
                    # ones column for the softmax denominator; no DRAM
                    # bounce, so the DMA stream carries no extra bytes.
                    xv_sb = acts.tile([BS, HD], F32, tag="xv_sb")
                    nc.vector.tensor_copy(xv_sb[:], xkv_ps[:, HD:])
                    vT_ps = ps_t.tile([128, BS], F32, tag="tr")
                    nc.tensor.transpose(vT_ps[:], xv_sb[:], ident[:BS, :BS])
                    vT_sb = acts.tile([128, BS], F32, tag="vT_sb")
                    nc.vector.tensor_copy(vT_sb[:], vT_ps[:])
                    v_new = acts.tile([S, B, VW], BF16, tag="v_new")
                    for b in range(B):
                        vn_ps = ps_t.tile([S, HD], F32, tag="tr")
                        nc.tensor.transpose(
                            vn_ps[:], vT_sb[:, 16 * b : 16 * (b + 1)],
                            ident[:],
                        )
                        nc.vector.tensor_copy(v_new[:, b, :HD], vn_ps[:])
                    nc.vector.memset(v_new[:, :, HD : HD + 1], 1.0)

                    for g in range(4):
                        for j in range(8):
                            c = 8 * g + j
                            nc.tensor.matmul(
                                xq_ps[:],
                                lhsT=xT_sb[:, c, :],
                                rhs=wq_tiles[g][:, j, :],
                                start=(c == 0),
                                stop=(c == KC - 1),
                            )

                    # ---- RoPE q (token-major): o = x*cc + swap(x)*ss ----
                    xq_sb = acts.tile([BS, QW], F32, tag="xq_sb")
                    tq = acts.tile([BS, QW], F32, tag="tq")
                    nc.vector.tensor_mul(xq_sb[:], xq_ps[:], ccq)
                    nc.vector.tensor_mul(tq[:], swap_pairs(xq_ps[:]), ssq)
                    nc.vector.tensor_add(xq_sb[:], xq_sb[:], tq[:])

                    # ---- transposes: kT_new [hd, (b,s)], qT [hd, (h | b,s)]
                    kTn_sb = acts.tile([128, BS], BF16, tag="kTn")
                    psn = ps_t.tile([128, BS], F32, tag="tr")
                    nc.tensor.transpose(psn[:], xk_sb[:], ident[:BS, :BS])
                    nc.vector.tensor_copy(kTn_sb[:], psn[:])
                    qT_sb = acts.tile([128, QH, BS], BF16, tag="qT")
                    for h in range(QH):
                        ps = ps_t.tile([128, BS], F32, tag="tr")
                        nc.tensor.transpose(
                            ps[:], xq_sb[:, 128 * h : 128 * (h + 1)],
                            ident[:BS, :BS],
                        )
                        nc.vector.tensor_copy(qT_sb[:, h, :], ps[:])

                # ---- attention per batch ----
                with (
                    tc.tile_pool(name="ps_sc", bufs=3, space="PSUM") as ps_sc,
                    tc.tile_pool(name="ps_ou", bufs=2, space="PSUM") as ps_ou,
                ):
                    # all four batches' K/V prefetched up front (kvp bufs=4),
                    # in consumption order ahead of the Wo blocks
                    kv_tiles = {}
                    for b in range(B):
                        kT_sb = kvp.tile([128, START], BF16, tag="kT",
                                         name="kT_sb")
                        nc.sync.dma_start(kT_sb[:], kT.ap()[b])
                        v_sb = kvp.tile([128, NT, VW], BF16, tag="v",
                                        name="v_sb")
                        nc.sync.dma_start(v_sb[:], vc.ap()[b])
                        kv_tiles[b] = (kT_sb, v_sb)

                    for b in range(B):
                        kT_sb, v_sb = kv_tiles.pop(b)

                        qT_b = qT_sb[:, :, 16 * b : 16 * (b + 1)]  # [128,4,16]

                        expT = expp.tile([128, EXPW], BF16, tag="expT")
                        for u in range(NT // 4):  # one exp per 4-chunk group
                            sc = ps_sc.tile([128, 4, TW], F32, tag="sc")
                            for j in range(4):
                                t = 4 * u + j
                                nc.tensor.matmul(
                                    sc[:, j, :],
                                    lhsT=kT_sb[:, 128 * t : 128 * (t + 1)],
                                    rhs=qT_b,
                                    start=True,
                                    stop=True,
                                )
                            nc.scalar.activation(
                                expT[:, 4 * TW * u : 4 * TW * (u + 1)],
                                sc[:],
                                mybir.ActivationFunctionType.Exp,
                                scale=SCALE,
                            )
                        scn = ps_sc.tile([S, TW], F32, tag="sc")
                        nc.tensor.matmul(
                            scn[:],
                            lhsT=kTn_sb[:, 16 * b : 16 * (b + 1)],
                            rhs=qT_b,
                            start=True,
                            stop=True,
                        )
                        nc.scalar.activation(
                            expT[:S, NT * TW :],
                            scn[:],
                            mybir.ActivationFunctionType.Exp,
                            scale=SCALE,
                        )

                        # unnormalized out [tok(h,s), hd | exp-sum col at HD]
                        ou = ps_ou.tile([TW, VW], F32, tag="ou")
                        for t in range(NT):
                            nc.tensor.matmul(
                                ou[:, : HD + 1],
                                lhsT=expT[:, TW * t : TW * (t + 1)],
                                rhs=v_sb[:, t, : HD + 1],
                                start=(t == 0),
                                stop=False,
                            )
                        nc.tensor.matmul(
                            ou[:, : HD + 1],
                            lhsT=expT[:S, NT * TW :],
                            rhs=v_new[:, b, : HD + 1],
                            start=False,
                            stop=True,
                        )

                        rcp = small.tile([TW, 1], F32, tag="rcp")
                        nc.vector.reciprocal(rcp[:], ou[:, HD : HD + 1])
                        attn = small.tile([TW, HD], F32, tag="attn")
                        nc.vector.tensor_scalar_mul(attn[:], ou[:, :HD], rcp[:])

                        aps = ps_t.tile([128, TW], F32, tag="tr")
                        nc.tensor.transpose(aps[:], attn[:], ident[:TW, :TW])
                        for h in range(QH):
                            nc.vector.tensor_copy(
                                attnT[:, h, 16 * b : 16 * (b + 1)],
                                aps[:, 16 * h : 16 * (h + 1)],
                            )

                        # Wo prefetch: blocks 0-3 as pairs, 4-5 as singles,
                        # 6-7 as 256KB halves — progressively finer toward
                        # the stream tail so the PE (whose matmuls per tile
                        # are faster than the tile's load) chases the loads
                        # with at most one sem-prop + one tile of lag
                        if b < 2:
                            wo_t = wop.tile([128, 2, 4, 512], BF16, tag="wo",
                                            name="wo_t")
                            nc.sync.dma_start(
                                wo_t[:], wo.ap()[:, 2 * b : 2 * (b + 1), :]
                            )
                            wo_sb.append(wo_t)
                        elif b == 2:
                            for j in (4, 5):
                                wo_t = wop.tile([128, 1, 4, 512], BF16,
                                                tag="wo1", name="wo_t")
                                nc.sync.dma_start(
                                    wo_t[:], wo.ap()[:, j : j + 1, :]
                                )
                                wo_sb.append(wo_t)
                        else:
                            wo5 = wo.ap().rearrange("p j (c n) -> p j c n", c=4)
                            for j in (6, 7):
                                for h in (0, 1):
                                    wo_t = wop2.tile([128, 4, 256], BF16,
                                                     tag="wo2", name="wo_t")
                                    nc.sync.dma_start(
                                        wo_t[:],
                                        wo5[:, j, :, 256 * h : 256 * (h + 1)],
                                    )
                                    wo_sb.append(wo_t)

            # ---- output projection: out[64, 4096] = attnT.T @ Wo ----
            # n-outer: out n-tile j consumes only column block j, so the Wo
            # matmuls chase the block loads
            with (
                tc.tile_pool(name="outp", bufs=6) as outp,
                tc.tile_pool(name="ps_wo", bufs=5, space="PSUM") as ps_wo,
            ):
                # (tile idx, sub idx, col offset in tile, out col, width,
                #  psum col offset or None for a fresh psum tile)
                pieces = [
                    (i // 2, i % 2, 0, 512 * i, 512, None) for i in range(4)
                ] + [
                    (2, 0, 0, 2048, 512, None), (3, 0, 0, 2560, 512, None),
                    (4, None, 0, 3072, 256, None), (5, None, 0, 3328, 256, None),
                    (6, None, 0, 3584, 256, None),
                    (7, None, 0, 3840, 128, None), (7, None, 128, 3968, 128, 128),
                ]
                # store groups: (piece indices, col0, width, engine 0=SP
                # 1=ACT). Early stores are wide and overlap the load
                # stream; the tail pieces merge into one 512-col store —
                # per-store overhead (625ns serialized HWDGE + ~650ns DGE)
                # exceeds the spread in the tail pieces' ready times. The
                # last two 128-col pieces share one psum tile and one DVE
                # copy (ACT handles the neighbouring tail copies).
                groups = [
                    ((0, 1), 0, 1024, 0), ((2, 3), 1024, 1024, 1),
                    ((4, 5, 6, 7), 2048, 1536, 0),
                    ((8, 9, 10), 3584, 512, 0),
                ]
                owner = {p: g for g, (ps, _, _, _) in enumerate(groups)
                         for p in ps}
                o_tiles = {}
                wo_ps = None
                for i, (ti, si, off, col, w, pso) in enumerate(pieces):
                    if pso is None:
                        wo_ps = ps_wo.tile([BS, 512], F32, tag="wops",
                                           name="wo_ps")
                        pso = 0
                    for k in range(QH):
                        rhs = (
                            wo_sb[ti][:, si, k, :]
                            if si is not None
                            else wo_sb[ti][:, k, off : off + w]
                        )
                        nc.tensor.matmul(
                            wo_ps[:, pso : pso + w],
                            lhsT=attnT[:, k, :],
                            rhs=rhs,
                            start=(k == 0),
                            stop=(k == QH - 1),
                        )
                    g = owner[i]
                    gps, gcol, gw, geng = groups[g]
                    if g not in o_tiles:
                        o_tiles[g] = outp.tile([BS, gw], BF16, tag="o",
                                               name="o_sb")
                    o_sb = o_tiles[g]
                    if i == 9:
                        continue  # copied together with piece 10
                    if i == 10:
                        nc.vector.tensor_copy(
                            o_sb[:, col - gcol - 128 : col - gcol + w],
                            wo_ps[:, : 128 + w],
                        )
                    elif i in (7, 8):
                        # balance the tail copies across ACT and DVE so the
                        # final store's wait ends as early as possible
                        nc.scalar.copy(
                            o_sb[:, col - gcol : col - gcol + w],
                            wo_ps[:, pso : pso + w],
                        )
                    else:
                        nc.vector.tensor_copy(
                            o_sb[:, col - gcol : col - gcol + w],
                            wo_ps[:, pso : pso + w],
                        )
                    if i == gps[-1]:
                        eng = nc.sync if geng == 0 else nc.scalar
                        eng.dma_start(
                            out.ap()[:, gcol : gcol + gw], o_sb[:]
                        )

    nc.compile()
    return nc


def _rope_tiles(freqs_cos, freqs_sin, nheads):
    # cc/ss broadcast tiles for token-major RoPE: row r=(b*16+s), col h*128+2i+j.
    # o = x*cc + swap(x)*ss with cc=[c,c,...], ss=[-s,+s,...].
    cos = np.asarray(freqs_cos, np.float32)  # [S, 64]
    sin = np.asarray(freqs_sin, np.float32)
    cc1 = np.repeat(cos, 2, axis=1)  # [S, 128]
    ss1 = np.repeat(sin, 2, axis=1).copy()
    ss1[:, 0::2] *= -1.0
    cc = np.tile(cc1, (B, nheads))  # [64, nheads*128]
    ss = np.tile(ss1, (B, nheads))
    return cc, ss


def _pmaj(w):
    # [KC*128, N] -> [128, KC*N] bf16: per-partition-contiguous SBUF order
    kc, n = w.shape[0] // 128, w.shape[1]
    return np.ascontiguousarray(
        w.reshape(kc, 128, n).transpose(1, 0, 2).reshape(128, kc * n)
        .astype(NPBF16)
    )


def _wo_blocks(w):
    # [512, 4096] -> [128, 8(block), 4(chunk)*512] bf16: column-block-major
    return np.ascontiguousarray(
        w.reshape(4, 128, 8, 512).transpose(1, 2, 0, 3).reshape(128, 8, 2048)
        .astype(NPBF16)
    )


def _v_pmaj(v):
    # [B, 2048, 128] -> [B, 128(p), NT*VW] bf16: chunk-row-major per
    # partition with a ones column at offset HD of each VW-wide chunk
    vb = v.reshape(B, NT, 128, HD).transpose(0, 2, 1, 3).astype(NPBF16)
    vp = np.zeros((B, 128, NT, VW), NPBF16)
    vp[:, :, :, :HD] = vb
    vp[:, :, :, HD] = NPBF16(1.0)
    return np.ascontiguousarray(vp.reshape(B, 128, NT * VW))


_BASS_CACHE = {}


def make_in_maps(x, freqs_cos, freqs_sin, cache_k, cache_v, Wq, Wk, Wv, Wo):
    x = np.asarray(x, np.float32)
    cache_k = np.asarray(cache_k, np.float32)
    cache_v = np.asarray(cache_v, np.float32)
    Wq = np.asarray(Wq, np.float32)
    Wk = np.asarray(Wk, np.float32)
    Wv = np.asarray(Wv, np.float32)
    Wo = np.asarray(Wo, np.float32)

    xT = np.ascontiguousarray(
        x.reshape(BS, KC, 128).transpose(2, 1, 0).reshape(128, KC * BS)
        .astype(NPBF16)
    )
    cck, ssk = _rope_tiles(freqs_cos, freqs_sin, 1)
    rope_cat = np.ascontiguousarray(
        np.concatenate([cck, ssk], axis=1).astype(NPBF16)
    )

    in_maps = []
    for c in range(NCORES):
        kc = cache_k[:, :START, c, :]  # [B, 2048, 128]
        kv = np.concatenate(
            [
                Wk[:, HD * c : HD * (c + 1)].reshape(KC, 128, HD),
                Wv[:, HD * c : HD * (c + 1)].reshape(KC, 128, HD),
            ],
            axis=2,
        ).reshape(KC * 128, 2 * HD)
        in_maps.append(
            {
                "xT": xT,
                "rope": rope_cat,
                "wq": _pmaj(Wq[:, QW * c : QW * (c + 1)]),
                "wkv": _pmaj(kv),
                "wo": _wo_blocks(Wo[QW * c : QW * (c + 1), :]),
                "kT": np.ascontiguousarray(
                    kc.transpose(0, 2, 1).astype(NPBF16)
                ),
                "vc": _v_pmaj(cache_v[:, :START, c, :]),
            }
        )
    return in_maps


def kernel(x, freqs_cos, freqs_sin, cache_k, cache_v, Wq, Wk, Wv, Wo, start_pos):
    assert int(start_pos) == START
    in_maps = make_in_maps(x, freqs_cos, freqs_sin, cache_k, cache_v, Wq, Wk, Wv, Wo)
    if "nc" not in _BASS_CACHE:
        _BASS_CACHE["nc"] = build_bass()
    res = run_bass_kernel_spmd(
        _BASS_CACHE["nc"], in_maps, core_ids=list(range(NCORES))
    )
    total = np.zeros((BS, DIM), np.float32)
    for r in res.results:
        total += np.asarray(r["out"], np.float32)
    return total.reshape(B, S, DIM)


# revision 63
# speedup vs baseline: 1.0032x; 1.0032x over previous
"""Trainium2 Bass kernel for single-step causal GQA attention with KV cache.

Problem (hardcoded shapes):
  x[4,16,4096] @ Wq/Wk/Wv -> RoPE -> append to KV cache (start_pos=2048) ->
  GQA attention over T=2064 keys -> @ Wo -> out[4,16,4096], fp32 in/out.

Sharding (8 cores, tensor-parallel over heads):
  core c gets q-heads 4c..4c+3 (Wq cols c*512:(c+1)*512), kv-head c
  (Wk/Wv cols c*128:(c+1)*128, cache [:, :, c, :]), and Wo rows
  c*512:(c+1)*512 (row-parallel). The 8 partial [64,4096] outputs are
  summed on the host.

The kernel is HBM-bound, so all large operands (weights, caches, x, the
attention intermediates, and the partial outputs) are bf16 — converted on
the host, halving DMA bytes. Accumulation stays fp32 in PSUM; measured
rel err ~1e-3 vs the 2e-2 gate.

Per-core layout choices:
  - x pre-transposed on host to xT[128, KC*64] (SBUF order, contiguous).
  - Wk|Wv interleaved per contraction chunk (wkv) so one matmul per chunk
    produces xk and xv side by side in one PSUM tile; wkv loads before wq
    so the new-token k/v path (RoPE-k, DRAM bounce of v) finishes early.
  - K cache pre-transposed to kT[4,128,2048]; score matmuls use kT chunks
    as stationary, scores come out transposed (keys on partitions).
  - V cache stored as 16 chunks of [128, 129] per batch with a ones
    column baked in at col 128 (host-side), so the softmax denominator
    accumulates next to the attention output in the same matmuls and the
    whole V load is one contiguous DMA.
  - softmax: exp without max-subtraction (|scores|<~10 here, exp exact in
    fp32), denominator via the ones column; no cross-partition reduction.
  - RoPE: two multiplies and an add with one host-precomputed [64,128]
    cos/sin pair (32KB load) serving k directly and q via a zero-stride
    AP repeating it across the 4 heads; pair-swap via negative-stride AP.
  - The new-token V is regrouped to per-batch layout with two PE
    transposes instead of a DRAM bounce (no extra stream bytes).
  - DMA count kept low (~30: each dma_start costs ~625ns on the
    serialized HWDGE path) and all loads issue on SP in consumption
    order so the DMA stream is gapless; Wo loads go progressively finer
    (pairs -> singles -> halves) so the tail of the stream has the
    shortest dependent chain, and the final out store is one small DMA
    whose wait covers only the last few columns.
"""

import math

import numpy as np
import ml_dtypes

import concourse.bass as bass
import concourse.mybir as mybir
import concourse.tile as tile
from concourse import bacc
from concourse.bass_utils import run_bass_kernel_spmd
from concourse.masks import make_identity

F32 = mybir.dt.float32
BF16 = mybir.dt.bfloat16
I16 = mybir.dt.int16
NPBF16 = ml_dtypes.bfloat16

B, S, DIM = 4, 16, 4096
NH, NKV, HD = 32, 8, 128
START = 2048
BS = B * S              # 64 tokens
NCORES = 8
QH = NH // NCORES       # 4 q heads per core
QW = QH * HD            # 512 = per-core Wq width
KC = DIM // 128         # 32 contraction chunks
NT = START // 128       # 16 full cache chunks per batch
TW = QH * S             # 64 = scoresT free width (h-major, then s)
EXPW = NT * TW + TW     # 1088 = expT tile width (16 cache chunks + new chunk)
VW = HD + 1             # 129 = V chunk stride (ones col at 128)
SCALE = 1.0 / math.sqrt(HD)


def swap_pairs(ap):
    # [p, w] view with even/odd element pairs swapped: (x1,x0,x3,x2,...)
    p, w = ap.shape
    return bass.AP(ap.tensor, ap.offset + 1, [ap.ap[0], [2, w // 2], [-1, 2]])


def build_bass() -> bass.Bass:
    nc = bacc.Bacc()

    xT = nc.dram_tensor("xT", [128, KC * BS], BF16, kind="ExternalInput")
    rope = nc.dram_tensor("rope", [BS, 2 * HD], BF16, kind="ExternalInput")
    wq = nc.dram_tensor("wq", [128, KC * QW], BF16, kind="ExternalInput")
    wkv = nc.dram_tensor("wkv", [128, KC * 2 * HD], BF16, kind="ExternalInput")
    kT = nc.dram_tensor("kT", [B, HD, START], BF16, kind="ExternalInput")
    vc = nc.dram_tensor("vc", [B, 128, NT * VW], BF16, kind="ExternalInput")
    wo = nc.dram_tensor("wo", [128, 8, 4 * 512], BF16, kind="ExternalInput")
    out = nc.dram_tensor("out", [BS, DIM], BF16, kind="ExternalOutput")

    with tile.TileContext(nc) as tc:
        with (
            tc.tile_pool(name="const", bufs=1) as const,
            tc.tile_pool(name="wqp", bufs=4) as wqp,
            tc.tile_pool(name="wkvp", bufs=1) as wkvp,
            tc.tile_pool(name="wop", bufs=2) as wop,
            tc.tile_pool(name="wop2", bufs=4) as wop2,
            tc.tile_pool(name="kvp", bufs=4) as kvp,
            tc.tile_pool(name="acts", bufs=1) as acts,
            tc.tile_pool(name="expp", bufs=2) as expp,
            tc.tile_pool(name="small", bufs=4) as small,
        ):
            ident = const.tile([128, 128], F32, tag="ident")
            make_identity(nc, ident[:])
            # touch Exp once so the ACT LUT loads during phase 1, not on
            # the batch-0 softmax critical path
            warm = const.tile([1, 4], F32, tag="warm")
            nc.scalar.activation(
                warm[:], ident[:1, :4], mybir.ActivationFunctionType.Exp
            )

            xT_sb = const.tile([128, KC, BS], BF16, tag="xT")
            rope_sb = const.tile([BS, 2 * HD], BF16, tag="rope")
            # one [64,128] cos/sin pair serves k directly and q via a
            # zero-stride AP that repeats it across the 4 heads
            cck = rope_sb[:, :HD]
            ssk = rope_sb[:, HD:]
            ccq = bass.AP(cck.tensor, cck.offset, [cck.ap[0], [0, QH], [1, HD]])
            ssq = bass.AP(ssk.tensor, ssk.offset, [ssk.ap[0], [0, QH], [1, HD]])

            wo_sb = []
            attnT = acts.tile([128, QH, BS], BF16, tag="attnT")

            with tc.tile_pool(name="ps_t", bufs=2, space="PSUM") as ps_t:
                # ---- QKV projections: psum[tok, width] += xT_c.T @ W_c ----
                with tc.tile_pool(name="ps_qkv", bufs=1, space="PSUM") as ps_qkv:
                    xq_ps = ps_qkv.tile([BS, QW], F32, tag="xq")
                    xkv_ps = ps_qkv.tile([BS, 2 * HD], F32, tag="xkv")

                    # Load order: wkv before wq, so the k/v new-token path
                    # (RoPE-k, DRAM bounce of v) completes early and never
                    # blocks the SP DMA queue; wq streams after.
                    x4 = xT.ap()
                    wq3 = wq.ap()
                    wkv2 = wkv.ap()
                    wkv_sb = wkvp.tile([128, KC, 2 * HD], BF16, tag="wkv")
                    # big wkv half first: its 2.9us transfer hides the SP
                    # issue cadence (~650ns/DMA) for the small loads behind
                    nc.sync.dma_start(wkv_sb[:, :16, :], wkv2[:, : 16 * 256])
                    nc.sync.dma_start(xT_sb[:, :8, :], x4[:, :512])
                    nc.sync.dma_start(rope_sb[:], rope.ap())
                    nc.sync.dma_start(xT_sb[:, 8:16, :], x4[:, 512:1024])
                    nc.sync.dma_start(xT_sb[:, 16:24, :], x4[:, 1024:1536])
                    nc.sync.dma_start(wkv_sb[:, 16:, :], wkv2[:, 16 * 256 :])
                    nc.sync.dma_start(xT_sb[:, 24:, :], x4[:, 1536:])
                    wq_tiles = []
                    for g in range(4):
                        wq_sb = wqp.tile([128, 8, QW], BF16, tag="wq")
                        nc.sync.dma_start(
                            wq_sb[:], wq3[:, 4096 * g : 4096 * (g + 1)]
                        )
                        wq_tiles.append(wq_sb)

                    for c in range(KC):
                        nc.tensor.matmul(
                            xkv_ps[:],
                            lhsT=xT_sb[:, c, :],
                            rhs=wkv_sb[:, c, :],
                            start=(c == 0),
                            stop=(c == KC - 1),
                        )

                    # ---- RoPE k + v_new bounce (early, off the SP queue) --
                    xk_sb = acts.tile([BS, HD], F32, tag="xk_sb")
                    tk = acts.tile([BS, HD], F32, tag="tk")
                    xk_ps = xkv_ps[:, :HD]
                    nc.vector.tensor_mul(xk_sb[:], xk_ps, cck)
                    nc.vector.tensor_mul(tk[:], swap_pairs(xk_ps), ssk)
                    nc.vector.tensor_add(xk_sb[:], xk_sb[:], tk[:])

                    # v_new: regroup the token-major new-token V to
                    # per-batch [16(s), b, HD] via a double PE transpose
                    # (token-major -> hd-major -> per-batch slices), with a
                    # ones column for the softmax denominator; no DRAM
                    # bounce, so the DMA stream carries no extra bytes.
                    xv_sb = acts.tile([BS, HD], F32, tag="xv_sb")
                    nc.vector.tensor_copy(xv_sb[:], xkv_ps[:, HD:])
                    vT_ps = ps_t.tile([128, BS], F32, tag="tr")
                    nc.tensor.transpose(vT_ps[:], xv_sb[:], ident[:BS, :BS])
                    vT_sb = acts.tile([128, BS], F32, tag="vT_sb")
                    nc.vector.tensor_copy(vT_sb[:], vT_ps[:])
                    v_new = acts.tile([S, B, VW], BF16, tag="v_new")
                    for b in range(B):
                        vn_ps = ps_t.tile([S, HD], F32, tag="tr")
                        nc.tensor.transpose(
                            vn_ps[:], vT_sb[:, 16 * b : 16 * (b + 1)],
                            ident[:],
                        )
                        nc.vector.tensor_copy(v_new[:, b, :HD], vn_ps[:])
                    nc.vector.memset(v_new[:, :, HD : HD + 1], 1.0)

                    for g in range(4):
                        for j in range(8):
                            c = 8 * g + j
                            nc.tensor.matmul(
                                xq_ps[:],
                                lhsT=xT_sb[:, c, :],
                                rhs=wq_tiles[g][:, j, :],
                                start=(c == 0),
                                stop=(c == KC - 1),
                            )

                    # ---- RoPE q (token-major): o = x*cc + swap(x)*ss ----
                    xq_sb = acts.tile([BS, QW], F32, tag="xq_sb")
                    tq = acts.tile([BS, QW], F32, tag="tq")
                    nc.vector.tensor_mul(xq_sb[:], xq_ps[:], ccq)
                    nc.vector.tensor_mul(tq[:], swap_pairs(xq_ps[:]), ssq)
                    nc.vector.tensor_add(xq_sb[:], xq_sb[:], tq[:])

                    # ---- transposes: kT_new [hd, (b,s)], qT [hd, (h | b,s)]
                    kTn_sb = acts.tile([128, BS], BF16, tag="kTn")
                    psn = ps_t.tile([128, BS], F32, tag="tr")
                    nc.tensor.transpose(psn[:], xk_sb[:], ident[:BS, :BS])
                    nc.vector.tensor_copy(kTn_sb[:], psn[:])
                    qT_sb = acts.tile([128, QH, BS], BF16, tag="qT")
                    for h in range(QH):
                        ps = ps_t.tile([128, BS], F32, tag="tr")
                        nc.tensor.transpose(
                            ps[:], xq_sb[:, 128 * h : 128 * (h + 1)],
                            ident[:BS, :BS],
                        )
                        nc.vector.tensor_copy(qT_sb[:, h, :], ps[:])

                # ---- attention per batch ----
                with (
                    tc.tile_pool(name="ps_sc", bufs=3, space="PSUM") as ps_sc,
                    tc.tile_pool(name="ps_ou", bufs=2, space="PSUM") as ps_ou,
                ):
                    # all four batches' K/V prefetched up front (kvp bufs=4),
                    # in consumption order ahead of the Wo blocks
                    kv_tiles = {}
                    for b in range(B):
                        kT_sb = kvp.tile([128, START], BF16, tag="kT",
                                         name="kT_sb")
                        nc.sync.dma_start(kT_sb[:], kT.ap()[b])
                        v_sb = kvp.tile([128, NT, VW], BF16, tag="v",
                                        name="v_sb")
                        nc.sync.dma_start(v_sb[:], vc.ap()[b])
                        kv_tiles[b] = (kT_sb, v_sb)

                    for b in range(B):
                        kT_sb, v_sb = kv_tiles.pop(b)

                        qT_b = qT_sb[:, :, 16 * b : 16 * (b + 1)]  # [128,4,16]

                        expT = expp.tile([128, EXPW], BF16, tag="expT")
                        for u in range(NT // 4):  # one exp per 4-chunk group
                            sc = ps_sc.tile([128, 4, TW], F32, tag="sc")
                            for j in range(4):
                                t = 4 * u + j
                                nc.tensor.matmul(
                                    sc[:, j, :],
                                    lhsT=kT_sb[:, 128 * t : 128 * (t + 1)],
                                    rhs=qT_b,
                                    start=True,
                                    stop=True,
                                )
                            nc.scalar.activation(
                                expT[:, 4 * TW * u : 4 * TW * (u + 1)],
                                sc[:],
                                mybir.ActivationFunctionType.Exp,
                                scale=SCALE,
                            )
                        scn = ps_sc.tile([S, TW], F32, tag="sc")
                        nc.tensor.matmul(
                            scn[:],
                            lhsT=kTn_sb[:, 16 * b : 16 * (b + 1)],
                            rhs=qT_b,
                            start=True,
                            stop=True,
                        )
                        nc.scalar.activation(
                            expT[:S, NT * TW :],
                            scn[:],
                            mybir.ActivationFunctionType.Exp,
                            scale=SCALE,
                        )

                        # unnormalized out [tok(h,s), hd | exp-sum col at HD]
                        ou = ps_ou.tile([TW, VW], F32, tag="ou")
                        for t in range(NT):
                            nc.tensor.matmul(
                                ou[:, : HD + 1],
                                lhsT=expT[:, TW * t : TW * (t + 1)],
                                rhs=v_sb[:, t, : HD + 1],
                                start=(t == 0),
                                stop=False,
                            )
                        nc.tensor.matmul(
                            ou[:, : HD + 1],
                            lhsT=expT[:S, NT * TW :],
                            rhs=v_new[:, b, : HD + 1],
                            start=False,
                            stop=True,
                        )

                        rcp = small.tile([TW, 1], F32, tag="rcp")
                        nc.vector.reciprocal(rcp[:], ou[:, HD : HD + 1])
                        attn = small.tile([TW, HD], F32, tag="attn")
                        nc.vector.tensor_scalar_mul(attn[:], ou[:, :HD], rcp[:])

                        aps = ps_t.tile([128, TW], F32, tag="tr")
                        nc.tensor.transpose(aps[:], attn[:], ident[:TW, :TW])
                        for h in range(QH):
                            nc.vector.tensor_copy(
                                attnT[:, h, 16 * b : 16 * (b + 1)],
                                aps[:, 16 * h : 16 * (h + 1)],
                            )

                        # Wo prefetch: blocks 0-3 as pairs, 4-5 as singles,
                        # 6-7 as 256KB halves — progressively finer toward
                        # the stream tail so the PE (whose matmuls per tile
                        # are faster than the tile's load) chases the loads
                        # with at most one sem-prop + one tile of lag
                        if b < 2:
                            wo_t = wop.tile([128, 2, 4, 512], BF16, tag="wo",
                                            name="wo_t")
                            nc.sync.dma_start(
                                wo_t[:], wo.ap()[:, 2 * b : 2 * (b + 1), :]
                            )
                            wo_sb.append(wo_t)
                        elif b == 2:
                            for j in (4, 5):
                                wo_t = wop.tile([128, 1, 4, 512], BF16,
                                                tag="wo1", name="wo_t")
                                nc.sync.dma_start(
                                    wo_t[:], wo.ap()[:, j : j + 1, :]
                                )
                                wo_sb.append(wo_t)
                        else:
                            wo5 = wo.ap().rearrange("p j (c n) -> p j c n", c=4)
                            for j in (6, 7):
                                for h in (0, 1):
                                    wo_t = wop2.tile([128, 4, 256], BF16,
                                                     tag="wo2", name="wo_t")
                                    if j == 7 and h == 1:
                                        # very last half split by contraction
                                        # chunk: the k=0,1 matmuls start one
                                        # 128KB transfer earlier, so only
                                        # two matmuls trail the last bytes
                                        for c in (0, 2):
                                            nc.sync.dma_start(
                                                wo_t[:, c : c + 2, :],
                                                wo5[:, j, c : c + 2,
                                                    256 * h : 256 * (h + 1)],
                                            )
                                    else:
                                        nc.sync.dma_start(
                                            wo_t[:],
                                            wo5[:, j, :,
                                                256 * h : 256 * (h + 1)],
                                        )
                                    wo_sb.append(wo_t)

            # ---- output projection: out[64, 4096] = attnT.T @ Wo ----
            # n-outer: out n-tile j consumes only column block j, so the Wo
            # matmuls chase the block loads
            with (
                tc.tile_pool(name="outp", bufs=6) as outp,
                tc.tile_pool(name="ps_wo", bufs=5, space="PSUM") as ps_wo,
            ):
                # (tile idx, sub idx, col offset in tile, out col, width,
                #  psum col offset or None for a fresh psum tile)
                pieces = [
                    (i // 2, i % 2, 0, 512 * i, 512, None) for i in range(4)
                ] + [
                    (2, 0, 0, 2048, 512, None), (3, 0, 0, 2560, 512, None),
                    (4, None, 0, 3072, 256, None), (5, None, 0, 3328, 256, None),
                    (6, None, 0, 3584, 256, None),
                    (7, None, 0, 3840, 128, None), (7, None, 128, 3968, 128, 128),
                ]
                # store groups: (piece indices, col0, width, engine 0=SP
                # 1=ACT). Early stores are wide and overlap the load
                # stream; the tail pieces merge into one 512-col store —
                # per-store overhead (625ns serialized HWDGE + ~650ns DGE)
                # exceeds the spread in the tail pieces' ready times. The
                # last two 128-col pieces share one psum tile and one DVE
                # copy (ACT handles the neighbouring tail copies).
                groups = [
                    ((0, 1), 0, 1024, 0), ((2, 3), 1024, 1024, 1),
                    ((4, 5, 6, 7), 2048, 1536, 0),
                    ((8, 9, 10), 3584, 512, 0),
                ]
                owner = {p: g for g, (ps, _, _, _) in enumerate(groups)
                         for p in ps}
                o_tiles = {}
                wo_ps = None
                for i, (ti, si, off, col, w, pso) in enumerate(pieces):
                    if pso is None:
                        wo_ps = ps_wo.tile([BS, 512], F32, tag="wops",
                                           name="wo_ps")
                        pso = 0
                    for k in range(QH):
                        rhs = (
                            wo_sb[ti][:, si, k, :]
                            if si is not None
                            else wo_sb[ti][:, k, off : off + w]
                        )
                        nc.tensor.matmul(
                            wo_ps[:, pso : pso + w],
                            lhsT=attnT[:, k, :],
                            rhs=rhs,
                            start=(k == 0),
                            stop=(k == QH - 1),
                        )
                    g = owner[i]
                    gps, gcol, gw, geng = groups[g]
                    if g not in o_tiles:
                        o_tiles[g] = outp.tile([BS, gw], BF16, tag="o",
                                               name="o_sb")
                    o_sb = o_tiles[g]
                    if i == 9:
                        continue  # copied together with piece 10
                    if i == 10:
                        nc.vector.tensor_copy(
                            o_sb[:, col - gcol - 128 : col - gcol + w],
                            wo_ps[:, : 128 + w],
                        )
                    elif i in (7, 8):
                        # balance the tail copies across ACT and DVE so the
                        # final store's wait ends as early as possible
                        nc.scalar.copy(
                            o_sb[:, col - gcol : col - gcol + w],
                            wo_ps[:, pso : pso + w],
                        )
                    else:
                        nc.vector.tensor_copy(
                            o_sb[:, col - gcol : col - gcol + w],
                            wo_ps[:, pso : pso + w],
                        )
                    if i == gps[-1]:
                        eng = nc.sync if geng == 0 else nc.scalar
                        eng.dma_start(
                            out.ap()[:, gcol : gcol + gw], o_sb[:]
                        )

    nc.compile()
    return nc


def _rope_tiles(freqs_cos, freqs_sin, nheads):
    # cc/ss broadcast tiles for token-major RoPE: row r=(b*16+s), col h*128+2i+j.
    # o = x*cc + swap(x)*ss with cc=[c,c,...], ss=[-s,+s,...].
    cos = np.asarray(freqs_cos, np.float32)  # [S, 64]
    sin = np.asarray(freqs_sin, np.float32)
    cc1 = np.repeat(cos, 2, axis=1)  # [S, 128]
    ss1 = np.repeat(sin, 2, axis=1).copy()
    ss1[:, 0::2] *= -1.0
    cc = np.tile(cc1, (B, nheads))  # [64, nheads*128]
    ss = np.tile(ss1, (B, nheads))
    return cc, ss


def _pmaj(w):
    # [KC*128, N] -> [128, KC*N] bf16: per-partition-contiguous SBUF order
    kc, n = w.shape[0] // 128, w.shape[1]
    return np.ascontiguousarray(
        w.reshape(kc, 128, n).transpose(1, 0, 2).reshape(128, kc * n)
        .astype(NPBF16)
    )


def _wo_blocks(w):
    # [512, 4096] -> [128, 8(block), 4(chunk)*512] bf16: column-block-major
    return np.ascontiguousarray(
        w.reshape(4, 128, 8, 512).transpose(1, 2, 0, 3).reshape(128, 8, 2048)
        .astype(NPBF16)
    )


def _v_pmaj(v):
    # [B, 2048, 128] -> [B, 128(p), NT*VW] bf16: chunk-row-major per
    # partition with a ones column at offset HD of each VW-wide chunk
    vb = v.reshape(B, NT, 128, HD).transpose(0, 2, 1, 3).astype(NPBF16)
    vp = np.zeros((B, 128, NT, VW), NPBF16)
    vp[:, :, :, :HD] = vb
    vp[:, :, :, HD] = NPBF16(1.0)
    return np.ascontiguousarray(vp.reshape(B, 128, NT * VW))


_BASS_CACHE = {}


def make_in_maps(x, freqs_cos, freqs_sin, cache_k, cache_v, Wq, Wk, Wv, Wo):
    x = np.asarray(x, np.float32)
    cache_k = np.asarray(cache_k, np.float32)
    cache_v = np.asarray(cache_v, np.float32)
    Wq = np.asarray(Wq, np.float32)
    Wk = np.asarray(Wk, np.float32)
    Wv = np.asarray(Wv, np.float32)
    Wo = np.asarray(Wo, np.float32)

    xT = np.ascontiguousarray(
        x.reshape(BS, KC, 128).transpose(2, 1, 0).reshape(128, KC * BS)
        .astype(NPBF16)
    )
    cck, ssk = _rope_tiles(freqs_cos, freqs_sin, 1)
    rope_cat = np.ascontiguousarray(
        np.concatenate([cck, ssk], axis=1).astype(NPBF16)
    )

    in_maps = []
    for c in range(NCORES):
        kc = cache_k[:, :START, c, :]  # [B, 2048, 128]
        kv = np.concatenate(
            [
                Wk[:, HD * c : HD * (c + 1)].reshape(KC, 128, HD),
                Wv[:, HD * c : HD * (c + 1)].reshape(KC, 128, HD),
            ],
            axis=2,
        ).reshape(KC * 128, 2 * HD)
        in_maps.append(
            {
                "xT": xT,
                "rope": rope_cat,
                "wq": _pmaj(Wq[:, QW * c : QW * (c + 1)]),
                "wkv": _pmaj(kv),
                "wo": _wo_blocks(Wo[QW * c : QW * (c + 1), :]),
                "kT": np.ascontiguousarray(
                    kc.transpose(0, 2, 1).astype(NPBF16)
                ),
                "vc": _v_pmaj(cache_v[:, :START, c, :]),
            }
        )
    return in_maps


def kernel(x, freqs_cos, freqs_sin, cache_k, cache_v, Wq, Wk, Wv, Wo, start_pos):
    assert int(start_pos) == START
    in_maps = make_in_maps(x, freqs_cos, freqs_sin, cache_k, cache_v, Wq, Wk, Wv, Wo)
    if "nc" not in _BASS_CACHE:
        _BASS_CACHE["nc"] = build_bass()
    res = run_bass_kernel_spmd(
        _BASS_CACHE["nc"], in_maps, core_ids=list(range(NCORES))
    )
    total = np.zeros((BS, DIM), np.float32)
    for r in res.results:
        total += np.asarray(r["out"], np.float32)
    return total.reshape(B, S, DIM)
